# revision 1
# baseline (speedup 1.0000x reference)
# Multi-head attention (b=2, n=2048, d_model=1024, 16 heads) on 8 NeuronCores.
#
# Sharding: core c = (batch b, head-group g) with b = c//4, g = c%4.
# Each core handles 1 batch element and 4 heads (256 channels), computing a
# partial output projection; the host sums the 4 group-partials per batch and
# adds b_O.
#
# Device layout (everything oriented so no transposes are needed):
#   xT   [D, N]      = x[b].T (bf16)             rhs of Q/K proj, lhsT of V
#   Q.T/K.T [4][128, N] per-head DUPLICATED rows (0:64 == 64:128), produced
#        by matmuls against host-duplicated weight columns.  The duplication
#        lets scores for m-slices i and i+1 run CONCURRENTLY as 64-row
#        tile_position pairs (rows 0-63 / 64-127 of the PE array).
#   V    [N, CH] natural layout (+b_v), stored per-head with an appended
#        ones column: lhsT [m, 65] so the O-matmul's PSUM row 64 accumulates
#        the softmax denominators for free.
#   S.T  [m-slice, n] per head = K_h @ Q_h.T     (K=64 contraction)
#   E.T  = exp(S.T / 8) on ScalarE (scale folded into the activation), bf16
#   O.T+sums [65, n] = [V_h | 1].T @ E.T         (accumulate over m-slices)
#   Y.T  [D, N] = woT.T @ (O.T * recip(sums)), DVE-copied out of PSUM, DMA.
#
# Matmul operands are bf16 (fp32 PSUM accumulation); fp32r measured ~3x
# slower on HW (cold-HAM equilibrium at ~630ns per 512-row matmul).

import ml_dtypes
import numpy as np

import concourse.bass as bass
import concourse.bacc as bacc
import concourse.tile as tile
from concourse import mybir
from concourse.bass_utils import run_bass_kernel_spmd

D = 1024  # d_model
N = 2048  # sequence length
B = 2  # batch
NHEADS = 16
DK = 64
NCORES = 8
GROUPS = 4  # head-groups across cores
HPG = NHEADS // GROUPS  # 4 heads per group
CH = HPG * DK  # 256 channels per group
KT = D // 128  # 8 contraction tiles for the projections
MS = N // 128  # 16 m-slices (key dim)
NCHUNK = 1024  # n-chunk width for the attention phase
NCHUNKS = N // NCHUNK

F32 = mybir.dt.float32
BF16 = mybir.dt.bfloat16


def _build_bass():
    nc = bacc.Bacc()

    xT_d = nc.dram_tensor("xT", [D, N], BF16, kind="ExternalInput")
    wqT_d = nc.dram_tensor("wqT", [D, CH], BF16, kind="ExternalInput")
    wkT_d = nc.dram_tensor("wkT", [D, CH], BF16, kind="ExternalInput")
    wvT_d = nc.dram_tensor("wvT", [D, CH], BF16, kind="ExternalInput")
    woT_d = nc.dram_tensor("woT", [CH, D], BF16, kind="ExternalInput")
    bq_d = nc.dram_tensor("bq", [CH], F32, kind="ExternalInput")
    bk_d = nc.dram_tensor("bk", [CH], F32, kind="ExternalInput")
    bv_d = nc.dram_tensor("bv", [CH], F32, kind="ExternalInput")
    vones_d = nc.dram_tensor("vones", [128, HPG], BF16, kind="ExternalInput")
    yT_d = nc.dram_tensor("yT", [D, N], F32, kind="ExternalOutput")

    with tile.TileContext(nc) as tc:
        with (
            tc.tile_pool(name="persist", bufs=1) as persist,
            tc.tile_pool(name="ph1", bufs=1) as ph1,
            tc.tile_pool(name="et_pool", bufs=4) as et_pool,
            tc.tile_pool(name="osb_pool", bufs=2) as osb_pool,
            tc.tile_pool(name="small", bufs=2) as small,
            tc.tile_pool(name="aux_ps", bufs=2, space="PSUM") as aux_ps,
            tc.tile_pool(name="st_ps", bufs=2, space="PSUM") as st_pool,
            tc.tile_pool(name="ot_ps", bufs=1, space="PSUM") as ot_pool,
        ):
            # ---- input loads, interleaved per k-tile so the first
            # ---- projection chains can start after ~2 tiles
            xt, wq, wk, wv = [], [], [], []
            for k in range(KT):
                t = ph1.tile([128, N], BF16, tag=f"xt{k}", name=f"xt{k}")
                nc.sync.dma_start(out=t, in_=xT_d[k * 128 : (k + 1) * 128, :])
                xt.append(t)
                for wname, dram, lst in (("wk", wkT_d, wk), ("wq", wqT_d, wq), ("wv", wvT_d, wv)):
                    t = ph1.tile([128, CH], BF16, tag=f"{wname}{k}", name=f"{wname}{k}")
                    nc.sync.dma_start(out=t, in_=dram[k * 128 : (k + 1) * 128, :])
                    lst.append(t)
            bq_t, bk_t = [], []
            for bname, dram, lst in (("bq", bq_d, bq_t), ("bk", bk_d, bk_t)):
                for cs in range(CH // 128):
                    t = ph1.tile([128, 1], F32, tag=f"{bname}{cs}", name=f"{bname}{cs}")
                    nc.sync.dma_start(out=t, in_=dram[cs * 128 : (cs + 1) * 128])
                    lst.append(t)
            bvb = ph1.tile([128, CH], F32, tag="bvb", name="bvb")
            bv_ap = bv_d[None, :]
            nc.gpsimd.dma_start(
                out=bvb,
                in_=bass.AP(tensor=bv_ap.tensor, offset=bv_ap.offset, ap=[[0, 128]] + list(bv_ap.ap[1:])),
            )

            # ---- persistent tensors ----
            qt = [persist.tile([128, N], BF16, tag=f"qt{cs}", name=f"qt{cs}") for cs in range(CH // 128)]
            kt = [persist.tile([128, N], BF16, tag=f"kt{cs}", name=f"kt{cs}") for cs in range(CH // 128)]
            v4 = [persist.tile([128, HPG * 65], BF16, tag=f"v4_{ms}", name=f"v4_{ms}") for ms in range(MS)]
            wot = []
            for cs in range(CH // 128):
                t = persist.tile([128, D], BF16, tag=f"wot{cs}", name=f"wot{cs}")
                nc.sync.dma_start(out=t, in_=woT_d[cs * 128 : (cs + 1) * 128, :])
                wot.append(t)

            # ---- filler emitters: one PSUM-chain each, paced into the
            # ---- attention loop so the PE eats them during ScalarE-bound spans
            def emit_v(ms):
                ps = aux_ps.tile([128, 512], F32, tag="aux", name="aux_ps_t")
                for k in range(KT):
                    nc.tensor.matmul(
                        ps[:, 0:CH],
                        xt[k][:, ms * 128 : (ms + 1) * 128],
                        wv[k],
                        start=(k == 0),
                        stop=(k == KT - 1),
                    )
                v4v = v4[ms].rearrange("p (h c) -> p h c", c=65)
                nc.sync.dma_start(out=v4v[:, :, 64:65], in_=vones_d[:, :])
                nc.vector.tensor_add(
                    out=v4v[:, :, 0:64],
                    in0=ps[:, 0:CH].rearrange("p (h c) -> p h c", c=64),
                    in1=bvb.rearrange("p (h c) -> p h c", c=64),
                )

            def emit_qk_chain(isq, cs, n0):
                dst, w, bias = (qt, wq, bq_t) if isq else (kt, wk, bk_t)
                ps = aux_ps.tile([128, 512], F32, tag="aux", name="aux_ps_t")
                for k in range(KT):
                    nc.tensor.matmul(
                        ps,
                        w[k][:, cs * 128 : (cs + 1) * 128],
                        xt[k][:, n0 : n0 + 512],
                        start=(k == 0),
                        stop=(k == KT - 1),
                    )
                nc.vector.tensor_scalar_add(
                    out=dst[cs][:, n0 : n0 + 512], in0=ps, scalar1=bias[cs]
                )

            def emit_f(chunk, msl, j):
                n0 = chunk * NCHUNK
                osb = osb_tiles[chunk]
                yp = aux_ps.tile([128, 512], F32, tag="aux", name="aux_yt_t")
                for cs in range(CH // 128):
                    nc.tensor.matmul(
                        yp,
                        wot[cs][:, msl * 128 : (msl + 1) * 128],
                        osb[cs][:, j : j + 512],
                        start=(cs == 0),
                        stop=(cs == CH // 128 - 1),
                    )
                ysb = small.tile([128, 512], F32, tag="ysb", name="ysb_t", bufs=4)
                nc.vector.tensor_copy(out=ysb, in_=yp)
                nc.sync.dma_start(
                    out=yT_d[msl * 128 : (msl + 1) * 128, n0 + j : n0 + j + 512],
                    in_=ysb,
                )

            # prelude: only what head 0 of chunk 0 needs immediately
            emit_qk_chain(True, 0, 0)      # q cs0 cols 0:512
            emit_qk_chain(True, 0, 512)    # q cs0 cols 512:1024 (chunk0 j=512)
            emit_qk_chain(False, 0, 0)     # k cs0 m-slices 0..3
            for ms in range(3):
                emit_v(ms)

            # deadline-paced fillers, one per m-slice iteration:
            # h0: remaining k-cs0 chains + V(3..15) just in time for O.T
            fillers = [
                lambda: emit_qk_chain(False, 0, 512),
                lambda: emit_v(3),
                lambda: emit_qk_chain(False, 0, 1024),
                lambda: emit_v(4),
                lambda: emit_qk_chain(False, 0, 1536),
            ] + [lambda ms=ms: emit_v(ms) for ms in range(5, MS)]
            # h1: cs1 chains (needed by h2) ; h2/h3: chunk-1 q columns
            fillers += [lambda n0=n0: emit_qk_chain(False, 1, n0) for n0 in range(0, N, 512)]
            fillers += [
                lambda: emit_qk_chain(True, 1, 0),
                lambda: emit_qk_chain(True, 1, 512),
            ]
            fillers += [None] * 8  # rest of h1 slots idle
            fillers += [
                lambda: emit_qk_chain(True, 0, 1024),
                lambda: emit_qk_chain(True, 1, 1024),
            ] + [None] * 14
            fillers += [
                lambda: emit_qk_chain(True, 0, 1536),
                lambda: emit_qk_chain(True, 1, 1536),
            ] + [None] * 14

            osb_tiles = {}

            # ---- attention + output projection ----
            for chunk in range(NCHUNKS):
                n0 = chunk * NCHUNK
                osb_tiles[chunk] = [
                    osb_pool.tile([128, NCHUNK], BF16, tag=f"osb{cs}", name=f"osb{cs}")
                    for cs in range(CH // 128)
                ]
                osb = osb_tiles[chunk]
                if chunk == 1:
                    fgroups = [(msl, j) for msl in range(D // 128) for j in range(0, NCHUNK, 512)]
                    for msl, j in fgroups[:10]:
                        fillers.append(lambda msl=msl, j=j: emit_f(0, msl, j))
                    tail_reserve = fgroups[10:]
                for h in range(HPG):
                    cs, r0 = h // 2, (h % 2) * 64
                    qt_h = qt[cs][r0 : r0 + 64, :]
                    kt_h = kt[cs][r0 : r0 + 64, :]
                    ot = ot_pool.tile([65, NCHUNK], F32, tag="ot", name="ot_t")
                    for ms in range(MS):
                        if fillers:
                            flr = fillers.pop(0)
                            if flr is not None:
                                flr()
                        st = st_pool.tile([128, NCHUNK], F32, tag="st", name="st_t")
                        for j in range(0, NCHUNK, 512):
                            nc.tensor.matmul(
                                st[:, j : j + 512],
                                kt_h[:, ms * 128 : (ms + 1) * 128],
                                qt_h[:, n0 + j : n0 + j + 512],
                                start=True,
                                stop=True,
                            )
                        et = et_pool.tile([128, NCHUNK], BF16, tag="et", name="et_t")
                        nc.scalar.activation(
                            out=et,
                            in_=st,
                            func=mybir.ActivationFunctionType.Exp,
                            scale=float(1.0 / np.sqrt(DK)),
                        )
                        lhsT = v4[ms][:, h * 65 : (h + 1) * 65]
                        for j in range(0, NCHUNK, 512):
                            nc.tensor.matmul(
                                ot[:, j : j + 512],
                                lhsT,
                                et[:, j : j + 512],
                                start=(ms == 0),
                                stop=(ms == MS - 1),
                            )
                    # drain ot to SBUF fast (frees the accumulator bank), then
                    # reciprocal of the denominators via a [128, 8] reshuffle,
                    # broadcast, and the normalize-multiply into osb.
                    oraw = small.tile([65, NCHUNK], F32, tag="oraw", name="oraw_t")
                    nc.vector.tensor_copy(out=oraw, in_=ot)
                    rcin = small.tile([128, NCHUNK // 128], F32, tag="rcin", name="rcin_t")
                    nc.sync.dma_start(out=rcin, in_=oraw[64:65, :])
                    rc = small.tile([128, NCHUNK // 128], F32, tag="rc", name="rc_t")
                    nc.vector.reciprocal(out=rc, in_=rcin)
                    rflat = small.tile([1, NCHUNK], F32, tag="rflat", name="rflat_t")
                    nc.sync.dma_start(out=rflat, in_=rc)
                    rb = small.tile([128, NCHUNK], F32, tag="rb", name="rb_t")
                    nc.gpsimd.partition_broadcast(rb, rflat)
                    nc.vector.tensor_mul(
                        out=osb[cs][r0 : r0 + 64, :], in0=oraw[0:64, :], in1=rb[0:64, :]
                    )
            # reserved chunk-0 groups keep the PE warm through the last
            # head's normalize chain, then chunk 1's output projection
            for msl, j in tail_reserve:
                emit_f(0, msl, j)
            for msl in range(D // 128):
                for j in range(0, NCHUNK, 512):
                    emit_f(1, msl, j)
    nc.compile()
    return nc


_NC = None


def _get_nc():
    global _NC
    if _NC is None:
        _NC = _build_bass()
    return _NC


def build_in_maps(inputs):
    x = np.asarray(inputs["x"], dtype=np.float32)
    W_Q = np.asarray(inputs["W_Q"], dtype=np.float32)
    W_K = np.asarray(inputs["W_K"], dtype=np.float32)
    W_V = np.asarray(inputs["W_V"], dtype=np.float32)
    W_O = np.asarray(inputs["W_O"], dtype=np.float32)
    b_Q = np.asarray(inputs["b_Q"], dtype=np.float32)
    b_K = np.asarray(inputs["b_K"], dtype=np.float32)
    b_V = np.asarray(inputs["b_V"], dtype=np.float32)

    in_maps = []
    for c in range(NCORES):
        b, g = divmod(c, GROUPS)
        sl = slice(g * CH, (g + 1) * CH)
        in_maps.append(
            {
                "xT": np.ascontiguousarray(x[b].T.astype(ml_dtypes.bfloat16)),
                "wqT": np.ascontiguousarray(W_Q[sl, :].T.astype(ml_dtypes.bfloat16)),
                "wkT": np.ascontiguousarray(W_K[sl, :].T.astype(ml_dtypes.bfloat16)),
                "wvT": np.ascontiguousarray(W_V[sl, :].T.astype(ml_dtypes.bfloat16)),
                "woT": np.ascontiguousarray(W_O[:, sl].T.astype(ml_dtypes.bfloat16)),
                "bq": np.ascontiguousarray(b_Q[sl]),
                "bk": np.ascontiguousarray(b_K[sl]),
                "bv": np.ascontiguousarray(b_V[sl]),
                "vones": np.ones((128, HPG), dtype=ml_dtypes.bfloat16),
            }
        )
    return in_maps


def kernel(**inputs):
    in_maps = build_in_maps(inputs)
    nc = _get_nc()
    res = run_bass_kernel_spmd(nc, in_maps, core_ids=list(range(NCORES)))

    b_O = np.asarray(inputs["b_O"], dtype=np.float32)
    out = np.zeros((B, N, D), dtype=np.float32)
    for c in range(NCORES):
        b = c // GROUPS
        out[b] += res.results[c]["yT"].T
    out += b_O
    return out



# revision 5
# speedup vs baseline: 1.2314x; 1.2314x over previous
# Multi-head attention (b=2, n=2048, d_model=1024, 16 heads) on 8 NeuronCores.
#
# Sharding: core c = (batch b, head-group g) with b = c//4, g = c%4.
# Each core handles 1 batch element and 4 heads (256 channels), computing a
# partial output projection; the host sums the 4 group-partials per batch and
# adds b_O.
#
# v2 design (scalar-exp-bound schedule, ~147us EXP floor):
#  - Heads processed in PAIRS (cs in {0,1}; rows 0:64 / 64:128 of qt/kt[cs]).
#    The two score matmuls of a pair have K=64 and auto-derive PE row-tiles
#    (0,0)/(64,0) from their base partitions -> they stream CONCURRENTLY.
#  - Query chunks of 512; st pair-packed [128, 1024] (h_even | h_odd) in PSUM,
#    double-buffered; ONE [128,1024] Exp per (pair, m-slice) on ScalarE with
#    the 1/8 scale folded in, output DIRECTLY in fp8e4.
#  - A*V runs in fp8 DoubleRow: Ko=2 packs consecutive m-slices, so each
#    matmul streams 2 slices worth of E (half the PE stream time of bf16).
#    V is stored fp8 as v4p[mp] = [128, (ko=2, h=4, 72)] with a ones column
#    at offset 64 (softmax denominators fall out of PSUM row 64 for free).
#    (fp8 on E/V measured 1.7e-2 rel err vs the 2e-2 gate in host sim;
#    projections/scores stay bf16 - fp8 there blows the budget.)
#  - Segments run PAIR-MAJOR (all 4 chunks of pair 0, then pair 1) so kt[1]
#    isn't needed until slot 64. Q/K/V/O projection chains are deadline-paced
#    fillers eating PE idle under the scalar-bound attention loop; the et ring
#    (8 groups) lets A*V lag fillers without stalling ScalarE.

import ml_dtypes
import numpy as np

import concourse.bass as bass
import concourse.bacc as bacc
import concourse.tile as tile
from concourse import mybir
from concourse.bass_utils import run_bass_kernel_spmd

D = 1024  # d_model
N = 2048  # sequence length
B = 2  # batch
NHEADS = 16
DK = 64
NCORES = 8
GROUPS = 4  # head-groups across cores
HPG = NHEADS // GROUPS  # 4 heads per group
CH = HPG * DK  # 256 channels per group
KT = D // 128  # 8 contraction tiles for the projections
MS = N // 128  # 16 m-slices (key dim)
MP = MS // 2  # 8 m-slice pairs (DoubleRow Ko=2)
NCHUNK = 512  # query-chunk width
NCHUNKS = N // NCHUNK
VPITCH = 72  # per-head pitch in v4p (65 used, pad so ko-stride % 16 == 0)

F32 = mybir.dt.float32
BF16 = mybir.dt.bfloat16
FP8 = mybir.dt.float8e4


def _build_bass():
    nc = bacc.Bacc()

    xT_d = nc.dram_tensor("xT", [D, N], BF16, kind="ExternalInput")
    wqT_d = nc.dram_tensor("wqT", [D, CH], BF16, kind="ExternalInput")
    wkT_d = nc.dram_tensor("wkT", [D, CH], BF16, kind="ExternalInput")
    wvT_d = nc.dram_tensor("wvT", [D, CH], BF16, kind="ExternalInput")
    woT_d = nc.dram_tensor("woT", [CH, D], BF16, kind="ExternalInput")
    bq_d = nc.dram_tensor("bq", [CH], F32, kind="ExternalInput")
    bk_d = nc.dram_tensor("bk", [CH], F32, kind="ExternalInput")
    bv_d = nc.dram_tensor("bv", [CH], F32, kind="ExternalInput")
    vones_d = nc.dram_tensor("vones", [128, 2 * HPG], FP8, kind="ExternalInput")
    yT_d = nc.dram_tensor("yT", [D, N], F32, kind="ExternalOutput")

    with tile.TileContext(nc) as tc:
        with (
            tc.tile_pool(name="persist", bufs=1) as persist,
            tc.tile_pool(name="et_pool", bufs=8) as et_pool,
            tc.tile_pool(name="osb_pool", bufs=1) as osb_pool,
            tc.tile_pool(name="small", bufs=2) as small,
            tc.tile_pool(name="aux_ps", bufs=2, space="PSUM") as aux_ps,
            tc.tile_pool(name="st_ps", bufs=2, space="PSUM") as st_pool,
            tc.tile_pool(name="ot_ps", bufs=1, space="PSUM") as ot_pool,
        ):
            # ---- input loads, interleaved per k-tile so the first
            # ---- projection chains can start after ~2 tiles
            xt, wq, wk, wv = [], [], [], []
            for k in range(KT):
                t = persist.tile([128, N], BF16, tag=f"xt{k}", name=f"xt{k}")
                nc.sync.dma_start(out=t, in_=xT_d[k * 128 : (k + 1) * 128, :])
                xt.append(t)
                for wname, dram, lst in (("wk", wkT_d, wk), ("wq", wqT_d, wq), ("wv", wvT_d, wv)):
                    t = persist.tile([128, CH], BF16, tag=f"{wname}{k}", name=f"{wname}{k}")
                    nc.sync.dma_start(out=t, in_=dram[k * 128 : (k + 1) * 128, :])
                    lst.append(t)
            bq_t, bk_t = [], []
            for bname, dram, lst in (("bq", bq_d, bq_t), ("bk", bk_d, bk_t)):
                for cs in range(CH // 128):
                    t = persist.tile([128, 1], F32, tag=f"{bname}{cs}", name=f"{bname}{cs}")
                    nc.sync.dma_start(out=t, in_=dram[cs * 128 : (cs + 1) * 128])
                    lst.append(t)
            bvb = persist.tile([128, CH], F32, tag="bvb", name="bvb")
            bv_ap = bv_d[None, :]
            nc.gpsimd.dma_start(
                out=bvb,
                in_=bass.AP(tensor=bv_ap.tensor, offset=bv_ap.offset, ap=[[0, 128]] + list(bv_ap.ap[1:])),
            )

            # ---- persistent tensors ----
            qt = [persist.tile([128, N], BF16, tag=f"qt{cs}", name=f"qt{cs}") for cs in range(CH // 128)]
            kt = [persist.tile([128, N], BF16, tag=f"kt{cs}", name=f"kt{cs}") for cs in range(CH // 128)]
            # v4p[mp]: fp8, layout [128, (ko=2, h=4, VPITCH)]; per head cols
            # h*VPITCH .. +64 = V channels, col 64 = ones (denominator trick)
            v4p = [persist.tile([128, 2 * HPG * VPITCH], FP8, tag=f"v4p{mp}", name=f"v4p{mp}") for mp in range(MP)]
            wot = []
            for cs in range(CH // 128):
                t = persist.tile([128, D], BF16, tag=f"wot{cs}", name=f"wot{cs}")
                nc.sync.dma_start(out=t, in_=woT_d[cs * 128 : (cs + 1) * 128, :])
                wot.append(t)
            osb = {}
            for c in range(NCHUNKS):
                for cs in range(CH // 128):
                    osb[(c, cs)] = osb_pool.tile(
                        [128, NCHUNK], BF16, tag=f"osb{c}_{cs}", name=f"osb{c}_{cs}"
                    )

            # ---- filler emitters (projection chains on aux PSUM) ----
            def emit_v(ms):
                mp, ko = divmod(ms, 2)
                ps = aux_ps.tile([128, 512], F32, tag="aux", name="aux_ps_t")
                for k in range(KT):
                    nc.tensor.matmul(
                        ps[:, 0:CH],
                        xt[k][:, ms * 128 : (ms + 1) * 128],
                        wv[k],
                        start=(k == 0),
                        stop=(k == KT - 1),
                    )
                v4v = v4p[mp].rearrange("p (k h s) -> p k h s", k=2, h=HPG)
                if ko == 0:
                    nc.sync.dma_start(out=v4v[:, :, :, 64:65], in_=vones_d[:, :])
                nc.vector.tensor_add(
                    out=v4v[:, ko, :, 0:64],
                    in0=ps[:, 0:CH].rearrange("p (h c) -> p h c", c=64),
                    in1=bvb.rearrange("p (h c) -> p h c", c=64),
                )

            def emit_qk_chain(isq, cs, n0):
                dst, w, bias = (qt, wq, bq_t) if isq else (kt, wk, bk_t)
                ps = aux_ps.tile([128, 512], F32, tag="aux", name="aux_ps_t")
                for k in range(KT):
                    nc.tensor.matmul(
                        ps,
                        w[k][:, cs * 128 : (cs + 1) * 128],
                        xt[k][:, n0 : n0 + 512],
                        start=(k == 0),
                        stop=(k == KT - 1),
                    )
                nc.vector.tensor_scalar_add(
                    out=dst[cs][:, n0 : n0 + 512], in0=ps, scalar1=bias[cs]
                )

            def emit_f(c, msl):
                yp = aux_ps.tile([128, 512], F32, tag="aux", name="aux_yt_t")
                for cs in range(CH // 128):
                    nc.tensor.matmul(
                        yp,
                        wot[cs][:, msl * 128 : (msl + 1) * 128],
                        osb[(c, cs)],
                        start=(cs == 0),
                        stop=(cs == CH // 128 - 1),
                    )
                ysb = small.tile([128, 512], F32, tag="ysb", name="ysb_t", bufs=4)
                nc.vector.tensor_copy(out=ysb, in_=yp)
                nc.sync.dma_start(
                    out=yT_d[msl * 128 : (msl + 1) * 128, c * NCHUNK : (c + 1) * NCHUNK],
                    in_=ysb,
                )

            # ---- prelude: just what segment 0 needs to start ----
            emit_qk_chain(False, 0, 0)  # kt[0] cols 0:512 (ms 0..3)
            emit_qk_chain(True, 0, 0)   # qt[0] cols 0:512 (chunk 0)

            # ---- deadline-paced fillers, emitted AFTER scores+exp of their
            # slot (so score matmuls always lead in PE queue order) and
            # BEFORE the A*V matmul of their m-slice pair (so v4p[mp] writes
            # precede the DoubleRow matmul that reads them - Tile derives
            # dependencies from program order).
            seg_fill = {
                0: {
                    0: [lambda: emit_v(0)],
                    1: [lambda: emit_v(1)],
                    2: [lambda: emit_v(2)],
                    3: [lambda: emit_v(3), lambda: emit_qk_chain(False, 0, 512)],
                    4: [lambda: emit_v(4)],
                    5: [lambda: emit_v(5)],
                    6: [lambda: emit_v(6)],
                    7: [lambda: emit_v(7), lambda: emit_qk_chain(False, 0, 1024)],
                    8: [lambda: emit_v(8)],
                    9: [lambda: emit_v(9)],
                    10: [lambda: emit_v(10)],
                    11: [lambda: emit_v(11), lambda: emit_qk_chain(False, 0, 1536)],
                    12: [lambda: emit_v(12)],
                    13: [lambda: emit_v(13)],
                    14: [lambda: emit_v(14)],
                    15: [lambda: emit_v(15), lambda: emit_qk_chain(True, 0, 512)],
                },
                1: {
                    1: [lambda: emit_qk_chain(True, 0, 1024)],
                    3: [lambda: emit_qk_chain(False, 1, 0)],
                    5: [lambda: emit_qk_chain(False, 1, 512)],
                    7: [lambda: emit_qk_chain(True, 0, 1536)],
                    9: [lambda: emit_qk_chain(False, 1, 1024)],
                    11: [lambda: emit_qk_chain(False, 1, 1536)],
                },
                2: {
                    1: [lambda: emit_qk_chain(True, 1, 0)],
                    5: [lambda: emit_qk_chain(True, 1, 512)],
                },
                3: {
                    1: [lambda: emit_qk_chain(True, 1, 1024)],
                    5: [lambda: emit_qk_chain(True, 1, 1536)],
                },
                4: {},
                5: {2 * msl: [lambda msl=msl: emit_f(0, msl)] for msl in range(D // 128)},
                6: {2 * msl: [lambda msl=msl: emit_f(1, msl)] for msl in range(D // 128)},
                7: {2 * msl: [lambda msl=msl: emit_f(2, msl)] for msl in range(D // 128)},
            }

            # ---- attention: pair-major segments ----
            seg = 0
            for cs in range(2):
                for c in range(NCHUNKS):
                    n0 = c * NCHUNK
                    fillers = seg_fill[seg]
                    ot = [
                        ot_pool.tile([65, NCHUNK], F32, tag=f"ot{hi}", name=f"ot{hi}_t")
                        for hi in range(2)
                    ]
                    for mp in range(MP):
                        et = et_pool.tile([128, 2048], FP8, tag="et", name="et_t")
                        for mi in range(2):
                            ms = 2 * mp + mi
                            st = st_pool.tile([128, 1024], F32, tag="st", name="st_t")
                            for hi in range(2):
                                r0 = hi * 64
                                nc.tensor.matmul(
                                    st[:, hi * 512 : (hi + 1) * 512],
                                    kt[cs][r0 : r0 + 64, ms * 128 : (ms + 1) * 128],
                                    qt[cs][r0 : r0 + 64, n0 : n0 + 512],
                                    start=True,
                                    stop=True,
                                )
                            nc.scalar.activation(
                                out=et[:, mi * 1024 : (mi + 1) * 1024],
                                in_=st,
                                func=mybir.ActivationFunctionType.Exp,
                                scale=float(1.0 / np.sqrt(DK)),
                            )
                            for f in fillers.get(2 * mp + mi, []):
                                f()
                        etv = et.rearrange("p (k n) -> p k n", k=2)
                        v4v = v4p[mp].rearrange("p (k s) -> p k s", k=2)
                        for hi in range(2):
                            h = 2 * cs + hi
                            nc.tensor.matmul(
                                ot[hi],
                                v4v[:, :, h * VPITCH : h * VPITCH + 65],
                                etv[:, :, hi * 512 : (hi + 1) * 512],
                                start=(mp == 0),
                                stop=(mp == MP - 1),
                                perf_mode=mybir.MatmulPerfMode.DoubleRow,
                            )
                    # normalize: drain ot, reciprocal of row-64 sums via a
                    # [128, 4] reshuffle, broadcast, multiply into osb rows.
                    for hi in range(2):
                        oraw = small.tile([65, NCHUNK], F32, tag="oraw", name="oraw_t")
                        nc.vector.tensor_copy(out=oraw, in_=ot[hi])
                        rcin = small.tile([128, NCHUNK // 128], F32, tag="rcin", name="rcin_t")
                        nc.sync.dma_start(out=rcin, in_=oraw[64:65, :])
                        rc = small.tile([128, NCHUNK // 128], F32, tag="rc", name="rc_t")
                        nc.vector.reciprocal(out=rc, in_=rcin)
                        rflat = small.tile([1, NCHUNK], F32, tag="rflat", name="rflat_t")
                        nc.sync.dma_start(out=rflat, in_=rc)
                        rb = small.tile([128, NCHUNK], F32, tag="rb", name="rb_t")
                        nc.gpsimd.partition_broadcast(rb, rflat)
                        nc.vector.tensor_mul(
                            out=osb[(c, cs)][hi * 64 : (hi + 1) * 64, :],
                            in0=oraw[0:64, :],
                            in1=rb[0:64, :],
                        )
                    seg += 1
            # epilogue: last chunk's output projection
            for msl in range(D // 128):
                emit_f(3, msl)
    nc.compile()
    return nc


_NC = None


def _get_nc():
    global _NC
    if _NC is None:
        _NC = _build_bass()
    return _NC


def build_in_maps(inputs):
    x = np.asarray(inputs["x"], dtype=np.float32)
    W_Q = np.asarray(inputs["W_Q"], dtype=np.float32)
    W_K = np.asarray(inputs["W_K"], dtype=np.float32)
    W_V = np.asarray(inputs["W_V"], dtype=np.float32)
    W_O = np.asarray(inputs["W_O"], dtype=np.float32)
    b_Q = np.asarray(inputs["b_Q"], dtype=np.float32)
    b_K = np.asarray(inputs["b_K"], dtype=np.float32)
    b_V = np.asarray(inputs["b_V"], dtype=np.float32)

    in_maps = []
    for core in range(NCORES):
        b, g = divmod(core, GROUPS)
        sl = slice(g * CH, (g + 1) * CH)
        in_maps.append(
            {
                "xT": np.ascontiguousarray(x[b].T.astype(ml_dtypes.bfloat16)),
                "wqT": np.ascontiguousarray(W_Q[sl, :].T.astype(ml_dtypes.bfloat16)),
                "wkT": np.ascontiguousarray(W_K[sl, :].T.astype(ml_dtypes.bfloat16)),
                "wvT": np.ascontiguousarray(W_V[sl, :].T.astype(ml_dtypes.bfloat16)),
                "woT": np.ascontiguousarray(W_O[:, sl].T.astype(ml_dtypes.bfloat16)),
                "bq": np.ascontiguousarray(b_Q[sl]),
                "bk": np.ascontiguousarray(b_K[sl]),
                "bv": np.ascontiguousarray(b_V[sl]),
                "vones": np.ones((128, 2 * HPG), dtype=ml_dtypes.float8_e4m3),
            }
        )
    return in_maps


def kernel(**inputs):
    in_maps = build_in_maps(inputs)
    nc = _get_nc()
    res = run_bass_kernel_spmd(nc, in_maps, core_ids=list(range(NCORES)))

    b_O = np.asarray(inputs["b_O"], dtype=np.float32)
    out = np.zeros((B, N, D), dtype=np.float32)
    for core in range(NCORES):
        b = core // GROUPS
        out[b] += res.results[core]["yT"].T
    out += b_O
    return out
